# revision 15
# baseline (speedup 1.0000x reference)
"""MoE ConditionalFeedForward kernel for 8 trn2 NeuronCores.

Strategy: pair tensor-parallelism over experts. Cores (2p, 2p+1) co-own
experts (2p, 2p+1): each core holds HALF of the intermediate dim (I/2=2816)
of w1/w3/w2 for BOTH experts and processes the pair's full token set, so
per-core work auto-balances within the pair. Phase-2 outputs are partial
sums over the I-halves; the two halves are summed on host (fp16 partials).

Routing is deduplicated: a token routed to the same expert in both slots is
computed once and scattered to both slots. Token columns are grouped
[expert u | expert v] with compile-time-fixed group sizes CU/CV (the max
"big"/"small" unique counts over pairs, big expert first), so one SPMD
program serves all 8 cores.

Device math per core (cap = CU+CV columns, D=2048, IH=2816):
  phase 1: h[i, c] = silu(sum_d w1T[d,i] xT[d,c]) * (sum_d w3T[d,i] xT[d,c])
           per expert group of columns (d on partitions, weights pre-packed)
  phase 2: ytp[d, c] = sum_{i in half} h[i, c] * w2[i, d]   (partial)

Each (expert, matrix) accumulation group gets its OWN PSUM tile: a matmul
with start=True invalidates the whole 2 KB PSUM bank region it touches, so
two groups must never share a bank. Everything streams as bf16 (PSUM
accumulates f32); phase-2 partials are written as fp16 (more mantissa than
bf16 at the same DMA cost).
"""

import numpy as np
import ml_dtypes

BF16 = ml_dtypes.bfloat16

# Problem dims (hardcoded per contract; kernel.py must be self-contained).
T, A, E, D, I = 1024, 2, 8, 2048, 5632
N_CORES = 8
IH = I // 2           # per-core intermediate rows
IBH = IH // 128       # 22 i-blocks per core
DB = D // 128         # 16 d-chunks (phase-1 contraction)
NPASS = 16            # phase-2 passes over D
PW = D // NPASS       # 128 d-cols per pass
NDC = PW // 128       # 1 psum d-block per pass
GSZ = 16              # phase-2 i-blocks per weight DMA (1 MB)

_BUILD_CACHE = {}


def _build(cu, cv):
    """Build + compile the shared per-core Bass program for group sizes cu/cv."""
    key = (cu, cv)
    if key in _BUILD_CACHE:
        return _BUILD_CACHE[key]

    import concourse.mybir as mybir
    import concourse.tile as tile
    from concourse import bacc

    dt = mybir.dt
    WDT = dt.bfloat16
    F32 = dt.float32
    F16 = dt.float16

    cap = cu + cv
    assert cap <= 512, f"cap {cap} > 512 unsupported"
    U1 = 4 * DB * 128     # phase-1 unit cols per i-block: [w1u, w1v, w3u, w3v]
    U2 = 2 * NDC * 128    # phase-2 unit cols per (pass, i-block): [u, v] x NDC

    nc = bacc.Bacc("TRN2", target_bir_lowering=False, debug=False,
                   num_devices=N_CORES)

    xgt = nc.dram_tensor("xgt", [128, DB * cap], WDT, kind="ExternalInput").ap()
    wp1 = nc.dram_tensor("wp1", [IBH, 128, U1], WDT, kind="ExternalInput").ap()
    wp2 = nc.dram_tensor("wp2", [NPASS, 128, IBH * U2], WDT,
                         kind="ExternalInput").ap()
    # outputs are yT partials ([D, cols]) so phase 2's PSUM [d_block, c] tiles
    # write out contiguously; host untransposes + sums the pair's halves.
    ytu = nc.dram_tensor("ytu", [D, cu], F16, kind="ExternalOutput").ap()
    ytv = nc.dram_tensor("ytv", [D, cv], F16, kind="ExternalOutput").ap()

    with tile.TileContext(nc) as tc:
        with (
            tc.tile_pool(name="xpool", bufs=1) as xpool,
            tc.tile_pool(name="w1pool", bufs=3) as w1pool,
            tc.tile_pool(name="w2pool", bufs=8) as w2pool,
            tc.tile_pool(name="hpool", bufs=1) as hpool,
            tc.tile_pool(name="spool", bufs=2) as spool,
            tc.tile_pool(name="opool", bufs=4) as opool,
        ):
            xg = xpool.tile([128, DB * cap], WDT)
            h = hpool.tile([128, IBH * cap], WDT)

            # ---- phase 1: h blocks ----
            with tc.tile_pool(name="psA", bufs=1, space="PSUM") as psA:
                # x chunked so the first matmuls don't wait on the whole
                # transfer; first w1 unit in quarters, interleaved with x.
                for b in range(IBH):
                    wt = w1pool.tile([128, U1], WDT, tag="w1")
                    if b == 0:
                        for q in range(4):
                            nc.sync.dma_start(wt[:, q * 2048:(q + 1) * 2048],
                                              wp1[0][:, q * 2048:(q + 1) * 2048])
                            xl, xh = (q * 4) * cap, min(16, q * 4 + 4) * cap
                            nc.sync.dma_start(xg[:, xl:xh], xgt[:, xl:xh])
                    else:
                        nc.sync.dma_start(wt[:], wp1[b])
                    p1u = psA.tile([128, cu], F32, tag="p1u")
                    p1v = psA.tile([128, cv], F32, tag="p1v")
                    p3u = psA.tile([128, cu], F32, tag="p3u")
                    p3v = psA.tile([128, cv], F32, tag="p3v")
                    for ps, mo, c0, cn in ((p1u, 0, 0, cu), (p1v, 1, cu, cv),
                                           (p3u, 2, 0, cu), (p3v, 3, cu, cv)):
                        base = mo * DB * 128
                        for do in range(DB):
                            nc.tensor.matmul(
                                ps[:], wt[:, base + do * 128:base + (do + 1) * 128],
                                xg[:, do * cap + c0:do * cap + c0 + cn],
                                start=(do == 0), stop=(do == DB - 1))
                    su = spool.tile([128, cu], F32, tag="su")
                    nc.scalar.activation(su[:], p1u[:],
                                         mybir.ActivationFunctionType.Silu)
                    nc.vector.tensor_mul(h[:, b * cap:b * cap + cu], su[:], p3u[:])
                    sv = spool.tile([128, cv], F32, tag="sv")
                    nc.scalar.activation(sv[:], p1v[:],
                                         mybir.ActivationFunctionType.Silu)
                    nc.vector.tensor_mul(h[:, b * cap + cu:(b + 1) * cap],
                                         sv[:], p3v[:])

            # ---- phase 2: ytp[d, c] partials, pass ph covers d-cols
            # [ph*PW, (ph+1)*PW); accumulate over all IBH i-blocks ----
            with tc.tile_pool(name="psB", bufs=4, space="PSUM") as psB:
                for ph in range(NPASS):
                    pu = [psB.tile([128, cu], F32, tag=f"pu{dc}",
                                   name=f"pu{dc}") for dc in range(NDC)]
                    pv = [psB.tile([128, cv], F32, tag=f"pv{dc}",
                                   name=f"pv{dc}") for dc in range(NDC)]
                    for b0 in range(0, IBH, GSZ):
                        nb = min(GSZ, IBH - b0)
                        wt2 = w2pool.tile([128, GSZ * U2], WDT, tag="w2")
                        nc.sync.dma_start(wt2[:, :nb * U2],
                                          wp2[ph][:, b0 * U2:(b0 + nb) * U2])
                        for s in range(nb):
                            b = b0 + s
                            for dc in range(NDC):
                                lo = s * U2 + dc * 128
                                nc.tensor.matmul(
                                    pu[dc][:], wt2[:, lo:lo + 128],
                                    h[:, b * cap:b * cap + cu],
                                    start=(b == 0), stop=(b == IBH - 1))
                            for dc in range(NDC):
                                lo = s * U2 + NDC * 128 + dc * 128
                                nc.tensor.matmul(
                                    pv[dc][:], wt2[:, lo:lo + 128],
                                    h[:, b * cap + cu:(b + 1) * cap],
                                    start=(b == 0), stop=(b == IBH - 1))
                    for dc in range(NDC):
                        r0 = ph * PW + dc * 128
                        ou = opool.tile([128, cu], F16, tag="ou")
                        nc.vector.tensor_copy(ou[:], pu[dc][:])
                        nc.scalar.dma_start(ytu[r0:r0 + 128, :], ou[:])
                        ov = opool.tile([128, cv], F16, tag="ov")
                        nc.vector.tensor_copy(ov[:], pv[dc][:])
                        nc.scalar.dma_start(ytv[r0:r0 + 128, :], ov[:])

    nc.compile()
    _BUILD_CACHE[key] = nc
    return nc


def _pack1(w1u, w1v, w3u, w3v):
    """Four [IH, D] half-matrices -> phase-1 stream [IBH, 128, 4*DB*128]:
    per i-block unit [di, (mat, do, i)] so each d-chunk's stationary tile is
    contiguous."""
    def pk(w):
        # [b*128(i), do*128(di)] -> [b, di, do, i]
        return w.reshape(IBH, 128, DB, 128).transpose(0, 3, 2, 1)
    st = np.stack([pk(w1u), pk(w1v), pk(w3u), pk(w3v)], axis=2)  # [b,di,4,do,i]
    return np.ascontiguousarray(st).reshape(IBH, 128, 4 * DB * 128)


def _pack2(w2u, w2v):
    """Two [IH, D] halves -> phase-2 stream [NPASS, 128, IBH*2*NDC*128]:
    per (pass, i-block) unit [i, (e, dc, di)]."""
    def pk(w):
        # [b*128(i), ph*NDC*128(d)] -> [ph, b, i, dc, di]
        return w.reshape(IBH, 128, NPASS, NDC, 128).transpose(2, 0, 1, 3, 4)
    st = np.stack([pk(w2u), pk(w2v)], axis=3)  # [ph, b, i, e, dc, di]
    return np.ascontiguousarray(st.transpose(0, 2, 1, 3, 4, 5)).reshape(
        NPASS, 128, IBH * 2 * NDC * 128)


def _prepare(inputs):
    """Host routing + packing. Returns (nc, in_maps, scatter_info)."""
    x = np.asarray(inputs["x"])
    idx = np.asarray(inputs["expert_indices"])
    w1 = np.asarray(inputs["w1"])
    w2 = np.asarray(inputs["w2"])
    w3 = np.asarray(inputs["w3"])
    t, a = idx.shape

    # unique tokens per expert (dedup: same expert in both slots -> one column)
    toks = [np.flatnonzero((idx == e).any(axis=1)) for e in range(E)]
    counts = np.array([len(s) for s in toks])

    # fixed pairs (0,1),(2,3),...; big expert first within each pair
    pairs = []
    for p in range(E // 2):
        e0, e1 = 2 * p, 2 * p + 1
        pairs.append((e0, e1) if counts[e0] >= counts[e1] else (e1, e0))
    cu = max(2, max(counts[u] for u, _ in pairs))
    cv = max(2, max(counts[v] for _, v in pairs))
    cap = cu + cv

    nc = _build(cu, cv)

    x_bf = x.astype(BF16)
    w1b, w2b, w3b = (w.astype(BF16) for w in (w1, w2, w3))
    in_maps = []
    for u, v in pairs:
        xgc = np.zeros((cap, D), BF16)
        xgc[0:counts[u]] = x_bf[toks[u]]
        xgc[cu:cu + counts[v]] = x_bf[toks[v]]
        # [c, d] -> [di, do, c]
        xgt = np.ascontiguousarray(
            xgc.T.reshape(DB, 128, cap).transpose(1, 0, 2)).reshape(128, DB * cap)
        for hf in range(2):
            sl = slice(hf * IH, (hf + 1) * IH)
            in_maps.append({
                "xgt": xgt,
                "wp1": _pack1(w1b[u][sl], w1b[v][sl], w3b[u][sl], w3b[v][sl]),
                "wp2": _pack2(w2b[u][sl], w2b[v][sl]),
            })

    return nc, in_maps, (t, a, idx, pairs, toks, cu, cv)


def _scatter(results, scatter_info):
    t, a, idx, pairs, toks, cu, cv = scatter_info
    out = np.zeros((t, a, D), np.float32)
    for p, (u, v) in enumerate(pairs):
        ra, rb = results[2 * p], results[2 * p + 1]
        for e, name in ((u, "ytu"), (v, "ytv")):
            y = (ra[name].astype(np.float32) + rb[name].astype(np.float32)).T
            te = toks[e]
            ti, ai = np.nonzero(idx == e)
            out[ti, ai] = y[np.searchsorted(te, ti)]
    return out


def kernel(**inputs):
    from concourse.bass_utils import run_bass_kernel_spmd

    nc, in_maps, scatter_info = _prepare(inputs)
    res = run_bass_kernel_spmd(nc, in_maps, core_ids=list(range(N_CORES)))
    return _scatter(res.results, scatter_info)
